# revision 2
# baseline (speedup 1.0000x reference)
"""Feature-pyramid ROIAlign (multi-level crop) on 8 TRN2 NeuronCores.

Strategy (data-parallel over ROIs, per the sharding hint):
- Host does the *routing*: per-proposal pyramid-level assignment, 11x11
  patch-window origins, and sparse bilinear weight matrices W [121, 196]
  (built with exact f32 replication of the reference's discontinuous
  level-assignment math; sampling coords are continuous so f32/f64
  round-off there is harmless).
- Proposals are sharded so every core gets an identical per-level slot
  sequence (the SPMD graph is identical across cores; all per-core
  differences ride in as data: window origins + weights).
- Device per proposal: one HWDGE DMA with register-offset (dynamic) AP
  gathers the [11,11,256] bf16 patch from the channels-last feature map
  into SBUF [121, 256]; two bf16 matmuls (contraction over the 121 cells)
  interpolate into PSUM [128, 196] f32; DVE/ACT copy PSUM->SBUF; batched
  ~800KB DMAs write the [M, 256, 196] f32 output shard.
"""
import os
import numpy as np
import ml_dtypes

# ---------------------------------------------------------------- constants
RPN_SCALES = (2.0, 4.0, 8.0, 16.0)
BASE_SIZES = (8.0, 16.0, 32.0, 64.0)
S = 14
S2 = S * S
PW = 11
CELLS = PW * PW
C = 256
MAP_HW = (256, 128, 64, 32)
N_CORES = 8
GRP = 8  # proposals per output-DMA group

LAST_EXEC_TIME_NS = None

_GRAPH_CACHE = {}


# ---------------------------------------------------------------- host math
def _route_and_weights(proposals):
    p = proposals.astype(np.float32)
    x0, y0, x1, y1 = p[:, 1], p[:, 2], p[:, 3], p[:, 4]
    sizes = np.sqrt((x1 - x0) * (y1 - y0))
    base = np.asarray(BASE_SIZES, dtype=np.float32)
    dist = np.abs(sizes[:, None] - base[None, :])
    lvl = np.argmin(dist, axis=1).astype(np.int32)

    N = p.shape[0]
    stride = np.asarray(RPN_SCALES, dtype=np.float32)[lvl]
    M = np.asarray(MAP_HW, dtype=np.int32)[lvl]

    fx0, fy0, fx1, fy1 = (c / stride for c in (x0, y0, x1, y1))
    bw = (fx1 - fx0) / np.float32(S)
    bh = (fy1 - fy0) / np.float32(S)
    grid = np.arange(S, dtype=np.float32) + np.float32(0.5)
    xs = fx0[:, None] + grid[None, :] * bw[:, None] - np.float32(0.5)
    ys = fy0[:, None] + grid[None, :] * bh[:, None] - np.float32(0.5)

    def split(coord, Mv):
        c0 = np.floor(coord)
        frac = coord - c0
        i0 = np.clip(c0.astype(np.int64), 0, Mv - 1).astype(np.int32)
        i1 = np.minimum(i0 + 1, Mv - 1).astype(np.int32)
        return i0, i1, frac.astype(np.float32)

    Mv = M[:, None]
    yi0, yi1, wy = split(ys, Mv)
    xi0, xi1, wx = split(xs, Mv)

    oy = np.clip(yi0.min(axis=1), 0, M - PW)
    ox = np.clip(xi0.min(axis=1), 0, M - PW)
    ly0, ly1 = yi0 - oy[:, None], yi1 - oy[:, None]
    lx0, lx1 = xi0 - ox[:, None], xi1 - ox[:, None]
    assert ly0.min() >= 0 and lx0.min() >= 0 and ly1.max() < PW and lx1.max() < PW, \
        "proposal spans >11 feature cells; patch window too small"

    ii = np.arange(S)
    nn = np.arange(N)[:, None]
    Wy = np.zeros((N, S, PW), dtype=np.float32)
    Wx = np.zeros((N, S, PW), dtype=np.float32)
    np.add.at(Wy, (nn, ii[None, :], ly0), 1.0 - wy)
    np.add.at(Wy, (nn, ii[None, :], ly1), wy)
    np.add.at(Wx, (nn, ii[None, :], lx0), 1.0 - wx)
    np.add.at(Wx, (nn, ii[None, :], lx1), wx)
    Wfull = np.einsum("niy,njx->nyxij", Wy, Wx).reshape(N, CELLS, S2)
    return lvl, oy.astype(np.int32), ox.astype(np.int32), Wfull


def _shard(lvl):
    """Slot assignment: per level, pad ids to a multiple of N_CORES, then core
    k takes every N_CORES-th padded slot. Returns (slot_gid [N_CORES, M],
    level_seq [M]) with identical level_seq across cores."""
    slot_gid = [[] for _ in range(N_CORES)]
    level_seq = []
    for l in range(4):
        ids = np.where(lvl == l)[0]
        if len(ids) == 0:
            continue
        pad = (-len(ids)) % N_CORES
        ids = np.concatenate([ids, np.repeat(ids[-1], pad)])
        per = len(ids) // N_CORES
        for k in range(N_CORES):
            slot_gid[k].extend(ids[k::N_CORES].tolist())
        level_seq.extend([l] * per)
    return np.asarray(slot_gid, dtype=np.int64), np.asarray(level_seq, dtype=np.int64)


# ---------------------------------------------------------------- device graph
def _build_graph(level_seq):
    import concourse.bass as bass
    import concourse.bacc as bacc
    import concourse.mybir as mybir
    import concourse.tile as tile

    M = len(level_seq)
    SP = (mybir.EngineType.SP,)
    nc = bacc.Bacc()
    fcl = [
        nc.declare_dram_parameter(f"fcl{l}", [MAP_HW[l], MAP_HW[l], C],
                                  mybir.dt.bfloat16, isOutput=False)
        for l in range(4)
    ]
    wmat = nc.declare_dram_parameter("wmat", [CELLS, M, S2], mybir.dt.bfloat16,
                                     isOutput=False)
    orig = nc.declare_dram_parameter("orig", [1, 2 * M], mybir.dt.int32,
                                     isOutput=False)
    out = nc.declare_dram_parameter("out", [M, C, S2], mybir.dt.float32,
                                    isOutput=True)
    out_t = out[:].rearrange("p c n -> c p n")

    n_groups = (M + GRP - 1) // GRP
    with tile.TileContext(nc) as tc:
        with (
            tc.tile_pool(name="small", bufs=1) as psmall,
            tc.tile_pool(name="wpool", bufs=3) as pwp,
            tc.tile_pool(name="patch", bufs=16) as pp,
            tc.tile_pool(name="outp", bufs=2) as po,
            tc.tile_pool(name="ps", bufs=4, space="PSUM") as ppsum,
        ):
            orig_t = psmall.tile([1, 2 * M], mybir.dt.int32)
            nc.sync.dma_start(orig_t[:], orig[:])
            for g in range(n_groups):
                a = g * GRP
                b = min(a + GRP, M)
                grp = b - a
                wt = pwp.tile([CELLS, grp * S2], mybir.dt.bfloat16, tag="wt")
                nc.sync.dma_start(
                    wt[:].rearrange("k (p n) -> k p n", p=grp),
                    wmat[:, a:b, :])
                outA = po.tile([128, grp * S2], mybir.dt.float32, tag="outA")
                outB = po.tile([128, grp * S2], mybir.dt.float32, tag="outB")
                for q in range(grp):
                    j = a + q
                    f = fcl[level_seq[j]]
                    oy = nc.values_load(orig_t[0:1, 2 * j:2 * j + 1], engines=SP,
                                        skip_runtime_bounds_check=True)
                    ox = nc.values_load(orig_t[0:1, 2 * j + 1:2 * j + 2], engines=SP,
                                        skip_runtime_bounds_check=True)
                    pt = pp.tile([CELLS, C], mybir.dt.bfloat16, tag="pt")
                    nc.sync.dma_start(
                        pt[:], f[bass.ds(oy, PW), bass.ds(ox, PW), :])
                    ps0 = ppsum.tile([128, S2], mybir.dt.float32, tag="ps0")
                    ps1 = ppsum.tile([128, S2], mybir.dt.float32, tag="ps1")
                    sl = slice(q * S2, (q + 1) * S2)
                    nc.tensor.matmul(ps0[:], pt[:, 0:128], wt[:, sl],
                                     start=True, stop=True)
                    nc.tensor.matmul(ps1[:], pt[:, 128:256], wt[:, sl],
                                     start=True, stop=True)
                    nc.vector.tensor_copy(outA[:, sl], ps0[:])
                    nc.scalar.copy(outB[:, sl], ps1[:])
                nc.sync.dma_start(out_t[0:128, a:b, :], outA[:])
                nc.sync.dma_start(out_t[128:256, a:b, :], outB[:])
    nc.finalize()
    return nc


# ---------------------------------------------------------------- entry point
def kernel(f0, f1, f2, f3, proposals):
    global LAST_EXEC_TIME_NS
    try:
        import profile_hook
        profile_hook.install()
    except Exception:
        pass
    from concourse.bass_utils import run_bass_kernel_spmd

    feats = (f0, f1, f2, f3)
    N = proposals.shape[0]
    lvl, oy, ox, Wfull = _route_and_weights(np.asarray(proposals))
    slot_gid, level_seq = _shard(lvl)
    M = slot_gid.shape[1]

    key = tuple(level_seq.tolist())
    if key not in _GRAPH_CACHE:
        _GRAPH_CACHE[key] = _build_graph(level_seq)
    nc = _GRAPH_CACHE[key]

    feats_cl = [
        np.ascontiguousarray(np.asarray(f)[0].transpose(1, 2, 0)).astype(
            ml_dtypes.bfloat16)
        for f in feats
    ]
    Wbf = Wfull.astype(ml_dtypes.bfloat16)

    in_maps = []
    for k in range(N_CORES):
        gids = slot_gid[k]
        wm = np.ascontiguousarray(Wbf[gids].transpose(1, 0, 2))  # [121, M, 196]
        og = np.empty((1, 2 * M), np.int32)
        og[0, 0::2] = oy[gids]
        og[0, 1::2] = ox[gids]
        im = {f"fcl{l}": feats_cl[l] for l in range(4)}
        im["wmat"] = wm
        im["orig"] = og
        in_maps.append(im)

    trace = os.environ.get("KERNEL_TRACE", "0") == "1"
    res = run_bass_kernel_spmd(nc, in_maps, list(range(N_CORES)), trace=trace)
    LAST_EXEC_TIME_NS = res.exec_time_ns

    out_full = np.zeros((N, C, S2), dtype=np.float32)
    for k in range(N_CORES):
        out_full[slot_gid[k]] = res.results[k]["out"]
    return out_full.reshape(N, C, S, S)


# revision 4
# speedup vs baseline: 1.9799x; 1.9799x over previous
"""Feature-pyramid ROIAlign (multi-level crop) on 8 TRN2 NeuronCores.

Strategy (data-parallel over ROIs, per the sharding hint):
- Host does the *routing*: per-proposal pyramid-level assignment, 11x11
  patch-window origins, and sparse bilinear weight matrices W [121, 196]
  (built with exact f32 replication of the reference's discontinuous
  level-assignment math; sampling coords are continuous so f32/f64
  round-off there is harmless).
- Proposals are sharded so every core gets an identical per-level slot
  sequence (the SPMD graph is identical across cores; all per-core
  differences ride in as data: window origins + weights).
- Device per proposal: one HWDGE DMA with register-offset (dynamic) AP
  gathers the [11,11,256] bf16 patch from the channels-last feature map
  into SBUF [121, 256]; two bf16 matmuls (contraction over the 121 cells)
  interpolate into PSUM [128, 196] f32; DVE/ACT copy PSUM->SBUF; batched
  ~800KB DMAs write the [M, 256, 196] f32 output shard.
"""
import os
import numpy as np
import ml_dtypes

# ---------------------------------------------------------------- constants
RPN_SCALES = (2.0, 4.0, 8.0, 16.0)
BASE_SIZES = (8.0, 16.0, 32.0, 64.0)
S = 14
S2 = S * S
PW = 11
CELLS = PW * PW
C = 256
MAP_HW = (256, 128, 64, 32)
N_CORES = 8
GRP = 8  # proposals per output-DMA group

LAST_EXEC_TIME_NS = None

_GRAPH_CACHE = {}


# ---------------------------------------------------------------- host math
def _route_and_weights(proposals):
    p = proposals.astype(np.float32)
    x0, y0, x1, y1 = p[:, 1], p[:, 2], p[:, 3], p[:, 4]
    sizes = np.sqrt((x1 - x0) * (y1 - y0))
    base = np.asarray(BASE_SIZES, dtype=np.float32)
    dist = np.abs(sizes[:, None] - base[None, :])
    lvl = np.argmin(dist, axis=1).astype(np.int32)

    N = p.shape[0]
    stride = np.asarray(RPN_SCALES, dtype=np.float32)[lvl]
    M = np.asarray(MAP_HW, dtype=np.int32)[lvl]

    fx0, fy0, fx1, fy1 = (c / stride for c in (x0, y0, x1, y1))
    bw = (fx1 - fx0) / np.float32(S)
    bh = (fy1 - fy0) / np.float32(S)
    grid = np.arange(S, dtype=np.float32) + np.float32(0.5)
    xs = fx0[:, None] + grid[None, :] * bw[:, None] - np.float32(0.5)
    ys = fy0[:, None] + grid[None, :] * bh[:, None] - np.float32(0.5)

    def split(coord, Mv):
        c0 = np.floor(coord)
        frac = coord - c0
        i0 = np.clip(c0.astype(np.int64), 0, Mv - 1).astype(np.int32)
        i1 = np.minimum(i0 + 1, Mv - 1).astype(np.int32)
        return i0, i1, frac.astype(np.float32)

    Mv = M[:, None]
    yi0, yi1, wy = split(ys, Mv)
    xi0, xi1, wx = split(xs, Mv)

    oy = np.clip(yi0.min(axis=1), 0, M - PW)
    ox = np.clip(xi0.min(axis=1), 0, M - PW)
    ly0, ly1 = yi0 - oy[:, None], yi1 - oy[:, None]
    lx0, lx1 = xi0 - ox[:, None], xi1 - ox[:, None]
    assert ly0.min() >= 0 and lx0.min() >= 0 and ly1.max() < PW and lx1.max() < PW, \
        "proposal spans >11 feature cells; patch window too small"

    ii = np.arange(S)
    nn = np.arange(N)[:, None]
    Wy = np.zeros((N, S, PW), dtype=np.float32)
    Wx = np.zeros((N, S, PW), dtype=np.float32)
    np.add.at(Wy, (nn, ii[None, :], ly0), 1.0 - wy)
    np.add.at(Wy, (nn, ii[None, :], ly1), wy)
    np.add.at(Wx, (nn, ii[None, :], lx0), 1.0 - wx)
    np.add.at(Wx, (nn, ii[None, :], lx1), wx)
    Wfull = np.einsum("niy,njx->nyxij", Wy, Wx).reshape(N, CELLS, S2)
    return lvl, oy.astype(np.int32), ox.astype(np.int32), Wfull


def _shard(lvl):
    """Slot assignment: per level, pad ids to a multiple of N_CORES, then core
    k takes every N_CORES-th padded slot. Returns (slot_gid [N_CORES, M],
    level_seq [M]) with identical level_seq across cores."""
    slot_gid = [[] for _ in range(N_CORES)]
    level_seq = []
    for l in range(4):
        ids = np.where(lvl == l)[0]
        if len(ids) == 0:
            continue
        pad = (-len(ids)) % N_CORES
        ids = np.concatenate([ids, np.repeat(ids[-1], pad)])
        per = len(ids) // N_CORES
        for k in range(N_CORES):
            slot_gid[k].extend(ids[k::N_CORES].tolist())
        level_seq.extend([l] * per)
    return np.asarray(slot_gid, dtype=np.int64), np.asarray(level_seq, dtype=np.int64)


# ---------------------------------------------------------------- device graph
def _build_graph(level_seq):
    import concourse.bass as bass
    import concourse.bacc as bacc
    import concourse.mybir as mybir
    import concourse.tile as tile

    M = len(level_seq)
    SP = (mybir.EngineType.SP,)
    ACT = (mybir.EngineType.Activation,)
    nc = bacc.Bacc()
    fcl = [
        nc.declare_dram_parameter(f"fcl{l}", [MAP_HW[l] * MAP_HW[l], C],
                                  mybir.dt.bfloat16, isOutput=False)
        for l in range(4)
    ]
    wmat = nc.declare_dram_parameter("wmat", [CELLS, M, S2], mybir.dt.bfloat16,
                                     isOutput=False)
    # orig: fused cell offsets (oy*W+ox), SP-issued slots first then ACT slots
    orig = nc.declare_dram_parameter("orig", [1, M], mybir.dt.int32,
                                     isOutput=False)
    out = nc.declare_dram_parameter("out", [M, C, S2], mybir.dt.float32,
                                    isOutput=True)
    out_t = out[:].rearrange("p c n -> c p n")

    n_groups = (M + GRP - 1) // GRP
    sp_ptr = 0
    act_ptr = sum((min((g + 1) * GRP, M) - g * GRP + 1) // 2
                  for g in range(n_groups))  # total SP slots
    with tile.TileContext(nc) as tc:
        with (
            tc.tile_pool(name="small", bufs=1) as psmall,
            tc.tile_pool(name="wpool", bufs=3) as pwp,
            tc.tile_pool(name="patch", bufs=24) as pp,
            tc.tile_pool(name="outp", bufs=3) as po,
            tc.tile_pool(name="ps", bufs=4, space="PSUM") as ppsum,
        ):
            orig_t = psmall.tile([1, M], mybir.dt.int32)
            nc.sync.dma_start(orig_t[:], orig[:])
            for g in range(n_groups):
                a = g * GRP
                b = min(a + GRP, M)
                grp = b - a
                n_sp = (grp + 1) // 2
                n_act = grp - n_sp
                wt = pwp.tile([CELLS, grp * S2], mybir.dt.bfloat16, tag="wt")
                nc.gpsimd.dma_start(
                    wt[:].rearrange("k (p n) -> k p n", p=grp),
                    wmat[:, a:b, :])
                _, vals_sp = nc.values_load_multi_w_load_instructions(
                    orig_t[0:1, sp_ptr:sp_ptr + n_sp], engines=SP,
                    skip_runtime_bounds_check=True)
                vals_act = ()
                if n_act:
                    _, vals_act = nc.values_load_multi_w_load_instructions(
                        orig_t[0:1, act_ptr:act_ptr + n_act], engines=ACT,
                        skip_runtime_bounds_check=True)
                sp_ptr += n_sp
                act_ptr += n_act
                outA = po.tile([128, grp * S2], mybir.dt.float32, tag="outA")
                outB = po.tile([128, grp * S2], mybir.dt.float32, tag="outB")
                pts = []
                for q in range(grp):
                    j = a + q
                    Wl = MAP_HW[level_seq[j]]
                    eng = nc.sync if q % 2 == 0 else nc.scalar
                    comb = vals_sp[q // 2] if q % 2 == 0 else vals_act[q // 2]
                    pt = pp.tile([CELLS, C], mybir.dt.bfloat16, tag="pt")
                    src = (fcl[level_seq[j]][bass.ds(comb, PW * Wl), :]
                           .rearrange("(y x) c -> y x c", x=Wl)[:, 0:PW, :])
                    eng.dma_start(pt[:], src)
                    pts.append(pt)
                for q0 in range(0, grp, 2):
                    pair = min(2, grp - q0)
                    psA = ppsum.tile([128, pair * S2], mybir.dt.float32, tag="psA")
                    psB = ppsum.tile([128, pair * S2], mybir.dt.float32, tag="psB")
                    for dq in range(pair):
                        q = q0 + dq
                        sl_w = slice(q * S2, (q + 1) * S2)
                        sl_p = slice(dq * S2, (dq + 1) * S2)
                        nc.tensor.matmul(psA[:, sl_p], pts[q][:, 0:128],
                                         wt[:, sl_w], start=True, stop=True)
                        nc.tensor.matmul(psB[:, sl_p], pts[q][:, 128:256],
                                         wt[:, sl_w], start=True, stop=True)
                    sl_o = slice(q0 * S2, (q0 + pair) * S2)
                    nc.vector.tensor_copy(outA[:, sl_o], psA[:])
                    nc.vector.tensor_copy(outB[:, sl_o], psB[:])
                nc.gpsimd.dma_start(out_t[0:128, a:b, :], outA[:])
                nc.gpsimd.dma_start(out_t[128:256, a:b, :], outB[:])
    nc.finalize()
    return nc


# ---------------------------------------------------------------- entry point
def kernel(f0, f1, f2, f3, proposals):
    global LAST_EXEC_TIME_NS
    try:
        import profile_hook
        profile_hook.install()
    except Exception:
        pass
    from concourse.bass_utils import run_bass_kernel_spmd

    feats = (f0, f1, f2, f3)
    N = proposals.shape[0]
    lvl, oy, ox, Wfull = _route_and_weights(np.asarray(proposals))
    slot_gid, level_seq = _shard(lvl)
    M = slot_gid.shape[1]

    key = tuple(level_seq.tolist())
    if key not in _GRAPH_CACHE:
        _GRAPH_CACHE[key] = _build_graph(level_seq)
    nc = _GRAPH_CACHE[key]

    feats_cl = [
        np.ascontiguousarray(np.asarray(f)[0].transpose(1, 2, 0)).astype(
            ml_dtypes.bfloat16).reshape(-1, C)
        for f in feats
    ]
    Wbf = Wfull.astype(ml_dtypes.bfloat16)

    # slot -> engine-major position in the fused-offset table (must mirror
    # the graph's per-group even/odd SP/ACT slot split)
    sp_slots, act_slots = [], []
    n_groups = (M + GRP - 1) // GRP
    for g in range(n_groups):
        a, b = g * GRP, min((g + 1) * GRP, M)
        for q in range(b - a):
            (sp_slots if q % 2 == 0 else act_slots).append(a + q)
    slot_order = np.asarray(sp_slots + act_slots, dtype=np.int64)

    in_maps = []
    for k in range(N_CORES):
        gids = slot_gid[k]
        wm = np.ascontiguousarray(Wbf[gids].transpose(1, 0, 2))  # [121, M, 196]
        comb = (oy[gids].astype(np.int64) * np.asarray(MAP_HW)[lvl[gids]]
                + ox[gids]).astype(np.int32)
        og = comb[slot_order].reshape(1, M)
        im = {f"fcl{l}": feats_cl[l] for l in range(4)}
        im["wmat"] = wm
        im["orig"] = og
        in_maps.append(im)

    trace = os.environ.get("KERNEL_TRACE", "0") == "1"
    res = run_bass_kernel_spmd(nc, in_maps, list(range(N_CORES)), trace=trace)
    LAST_EXEC_TIME_NS = res.exec_time_ns

    out_full = np.zeros((N, C, S2), dtype=np.float32)
    for k in range(N_CORES):
        out_full[slot_gid[k]] = res.results[k]["out"]
    return out_full.reshape(N, C, S, S)


# revision 5
# speedup vs baseline: 2.0921x; 1.0567x over previous
"""Feature-pyramid ROIAlign (multi-level crop) on 8 TRN2 NeuronCores — v3.

Host does routing (level assignment, 11x11 windows, bilinear weight
matrices, int16 gather-index tables); device gathers patches with
dma_gather (one SWDGE instruction per ~8 proposals, cell-on-partition
matmul-ready layout), interpolates via bf16 matmuls (k=121 cells), and
writes the c-major f32 output shard with long-descriptor DMAs.

Proposals are sharded so all 8 cores share one SPMD graph: per-core
differences are data only (indices + weights). Level-0 gathers are split
into 3 overlapping row-bands of f0 so local cell ids fit int16.
"""
import os
import numpy as np
import ml_dtypes

RPN_SCALES = (2.0, 4.0, 8.0, 16.0)
BASE_SIZES = (8.0, 16.0, 32.0, 64.0)
S = 14
S2 = S * S
PW = 11
CELLS = PW * PW
C = 256
MAP_HW = (256, 128, 64, 32)
N_CORES = 8
GRP = 8

# regions: (level, row0, row1) — arena slice rows [row0, row1) of that level's
# map; level-0 split into 3 bands so (row1-row0)*W <= 32767 (int16 indices)
REGIONS = (
    (0, 0, 127), (0, 116, 243), (0, 232, 256),
    (1, 0, 128), (2, 0, 64), (3, 0, 32),
)

LAST_EXEC_TIME_NS = None
_GRAPH_CACHE = {}


def _route_and_weights(proposals):
    p = proposals.astype(np.float32)
    x0, y0, x1, y1 = p[:, 1], p[:, 2], p[:, 3], p[:, 4]
    sizes = np.sqrt((x1 - x0) * (y1 - y0))
    base = np.asarray(BASE_SIZES, dtype=np.float32)
    dist = np.abs(sizes[:, None] - base[None, :])
    lvl = np.argmin(dist, axis=1).astype(np.int32)

    N = p.shape[0]
    stride = np.asarray(RPN_SCALES, dtype=np.float32)[lvl]
    M = np.asarray(MAP_HW, dtype=np.int32)[lvl]

    fx0, fy0, fx1, fy1 = (c / stride for c in (x0, y0, x1, y1))
    bw = (fx1 - fx0) / np.float32(S)
    bh = (fy1 - fy0) / np.float32(S)
    grid = np.arange(S, dtype=np.float32) + np.float32(0.5)
    xs = fx0[:, None] + grid[None, :] * bw[:, None] - np.float32(0.5)
    ys = fy0[:, None] + grid[None, :] * bh[:, None] - np.float32(0.5)

    def split(coord, Mv):
        c0 = np.floor(coord)
        frac = coord - c0
        i0 = np.clip(c0.astype(np.int64), 0, Mv - 1).astype(np.int32)
        i1 = np.minimum(i0 + 1, Mv - 1).astype(np.int32)
        return i0, i1, frac.astype(np.float32)

    Mv = M[:, None]
    yi0, yi1, wy = split(ys, Mv)
    xi0, xi1, wx = split(xs, Mv)

    oy = np.clip(yi0.min(axis=1), 0, M - PW)
    ox = np.clip(xi0.min(axis=1), 0, M - PW)
    ly0, ly1 = yi0 - oy[:, None], yi1 - oy[:, None]
    lx0, lx1 = xi0 - ox[:, None], xi1 - ox[:, None]
    assert ly0.min() >= 0 and lx0.min() >= 0 and ly1.max() < PW and lx1.max() < PW, \
        "proposal spans >11 feature cells; patch window too small"

    ii = np.arange(S)
    nn = np.arange(N)[:, None]
    Wy = np.zeros((N, S, PW), dtype=np.float32)
    Wx = np.zeros((N, S, PW), dtype=np.float32)
    np.add.at(Wy, (nn, ii[None, :], ly0), 1.0 - wy)
    np.add.at(Wy, (nn, ii[None, :], ly1), wy)
    np.add.at(Wx, (nn, ii[None, :], lx0), 1.0 - wx)
    np.add.at(Wx, (nn, ii[None, :], lx1), wx)
    Wfull = np.einsum("niy,njx->nyxij", Wy, Wx).reshape(N, CELLS, S2)
    return lvl, oy.astype(np.int32), ox.astype(np.int32), Wfull


def _region_of(lvl, oy):
    """Region index per proposal (level-0 split by window row band)."""
    r = np.empty(lvl.shape[0], np.int64)
    for i in range(lvl.shape[0]):
        if lvl[i] == 0:
            if oy[i] <= 116:
                r[i] = 0
            elif oy[i] <= 232:
                r[i] = 1
            else:
                r[i] = 2
        else:
            r[i] = 2 + lvl[i]
    return r


def _shard(region):
    """Per region: pad ids to a multiple of N_CORES, core k takes k::N_CORES.
    Returns slot_gid [N_CORES, M] and region_seq [M] (same for all cores)."""
    slot_gid = [[] for _ in range(N_CORES)]
    region_seq = []
    for r in range(len(REGIONS)):
        ids = np.where(region == r)[0]
        if len(ids) == 0:
            continue
        pad = (-len(ids)) % N_CORES
        ids = np.concatenate([ids, np.repeat(ids[-1], pad)])
        per = len(ids) // N_CORES
        for k in range(N_CORES):
            slot_gid[k].extend(ids[k::N_CORES].tolist())
        region_seq.extend([r] * per)
    return (np.asarray(slot_gid, dtype=np.int64),
            np.asarray(region_seq, dtype=np.int64))


def _build_graph(region_seq):
    import concourse.bass as bass
    import concourse.bacc as bacc
    import concourse.mybir as mybir
    import concourse.tile as tile

    M = len(region_seq)
    nc = bacc.Bacc()
    fcl = [
        nc.declare_dram_parameter(f"fcl{l}", [MAP_HW[l] * MAP_HW[l], C],
                                  mybir.dt.bfloat16, isOutput=False)
        for l in range(4)
    ]
    arena = []
    for (l, r0, r1) in REGIONS:
        W = MAP_HW[l]
        arena.append(fcl[l][r0 * W:r1 * W, :])
    wmat = nc.declare_dram_parameter("wmat", [CELLS, M, S2], mybir.dt.bfloat16,
                                     isOutput=False)
    idx = nc.declare_dram_parameter("idx", [128, M * 8], mybir.dt.int16,
                                    isOutput=False)
    out = nc.declare_dram_parameter("out", [C, M, S2], mybir.dt.float32,
                                    isOutput=True)

    n_groups = (M + GRP - 1) // GRP
    with tile.TileContext(nc) as tc:
        with (
            tc.tile_pool(name="small", bufs=1) as psmall,
            tc.tile_pool(name="wpool", bufs=3) as pwp,
            tc.tile_pool(name="patch", bufs=4) as pp,
            tc.tile_pool(name="outp", bufs=3) as po,
            tc.tile_pool(name="ps", bufs=4, space="PSUM") as ppsum,
        ):
            idx_t = psmall.tile([128, M * 8], mybir.dt.int16)
            nc.sync.dma_start(idx_t[:], idx[:])
            for g in range(n_groups):
                a = g * GRP
                b = min(a + GRP, M)
                grp = b - a
                wt = pwp.tile([CELLS, grp * S2], mybir.dt.bfloat16, tag="wt")
                nc.sync.dma_start(
                    wt[:].rearrange("k (p n) -> k p n", p=grp),
                    wmat[:, a:b, :])
                pt = pp.tile([128, grp * C], mybir.dt.bfloat16, tag="pt")
                # one dma_gather per contiguous same-region run of slots
                q0 = 0
                while q0 < grp:
                    r = region_seq[a + q0]
                    q1 = q0
                    while q1 < grp and region_seq[a + q1] == r:
                        q1 += 1
                    run = q1 - q0
                    nc.gpsimd.dma_gather(
                        out_ap=pt[:, q0 * C:q1 * C].rearrange(
                            "p (q c) -> p q c", c=C),
                        in_ap=arena[r],
                        idxs_ap=idx_t[:, (a + q0) * 8:(a + q1) * 8],
                        num_idxs=run * 128,
                        num_idxs_reg=run * 128,
                        elem_size=C,
                    )
                    q0 = q1
                outA = po.tile([128, grp * S2], mybir.dt.float32, tag="outA")
                outB = po.tile([128, grp * S2], mybir.dt.float32, tag="outB")
                for q0 in range(0, grp, 2):
                    pair = min(2, grp - q0)
                    psA = ppsum.tile([128, pair * S2], mybir.dt.float32, tag="psA")
                    psB = ppsum.tile([128, pair * S2], mybir.dt.float32, tag="psB")
                    for dq in range(pair):
                        q = q0 + dq
                        sl_w = slice(q * S2, (q + 1) * S2)
                        sl_p = slice(dq * S2, (dq + 1) * S2)
                        nc.tensor.matmul(psA[:, sl_p],
                                         pt[0:CELLS, q * C:q * C + 128],
                                         wt[:, sl_w], start=True, stop=True)
                        nc.tensor.matmul(psB[:, sl_p],
                                         pt[0:CELLS, q * C + 128:(q + 1) * C],
                                         wt[:, sl_w], start=True, stop=True)
                    sl_o = slice(q0 * S2, (q0 + pair) * S2)
                    if q0 % 4 == 0:
                        nc.vector.tensor_copy(outA[:, sl_o], psA[:])
                        nc.vector.tensor_copy(outB[:, sl_o], psB[:])
                    else:
                        nc.vector.tensor_copy(outA[:, sl_o], psA[:])
                        nc.scalar.copy(outB[:, sl_o], psB[:])
                nc.scalar.dma_start(out[0:128, a:b, :], outA[:])
                nc.scalar.dma_start(out[128:256, a:b, :], outB[:])
    nc.finalize()
    return nc


def _prep_core_inputs(k, slot_gid, region_seq, lvl, oy, ox, Wbf):
    M = slot_gid.shape[1]
    gids = slot_gid[k]
    wm = np.ascontiguousarray(Wbf[gids].transpose(1, 0, 2))  # [121, M, 196]
    # gather index table: position i = j*128 + c -> idx[i%16, i//16]
    dy = np.repeat(np.arange(PW), PW)
    dx = np.tile(np.arange(PW), PW)
    cells = np.empty((M, 128), np.int64)
    for j in range(M):
        g = gids[j]
        l, r0, _ = REGIONS[region_seq[j]]
        W = MAP_HW[l]
        loc = (oy[g] - r0 + dy) * W + (ox[g] + dx)
        cells[j, :CELLS] = loc
        cells[j, CELLS:] = loc[-1]
    assert cells.min() >= 0 and cells.max() <= 32767
    flat = cells.reshape(-1).astype(np.int16)  # position-major
    base = flat.reshape(M * 8, 16).T  # [16, M*8]
    idx_tile = np.ascontiguousarray(np.tile(base, (8, 1)))  # 8 Q7 replicas
    return wm, idx_tile


def kernel(f0, f1, f2, f3, proposals):
    global LAST_EXEC_TIME_NS
    try:
        import profile_hook
        profile_hook.install()
    except Exception:
        pass
    from concourse.bass_utils import run_bass_kernel_spmd

    feats = (f0, f1, f2, f3)
    N = proposals.shape[0]
    lvl, oy, ox, Wfull = _route_and_weights(np.asarray(proposals))
    region = _region_of(lvl, oy)
    slot_gid, region_seq = _shard(region)
    M = slot_gid.shape[1]

    key = tuple(region_seq.tolist())
    if key not in _GRAPH_CACHE:
        _GRAPH_CACHE[key] = _build_graph(region_seq)
    nc = _GRAPH_CACHE[key]

    feats_cl = [
        np.ascontiguousarray(np.asarray(f)[0].transpose(1, 2, 0)).astype(
            ml_dtypes.bfloat16).reshape(-1, C)
        for f in feats
    ]
    Wbf = Wfull.astype(ml_dtypes.bfloat16)

    in_maps = []
    for k in range(N_CORES):
        wm, idx_tile = _prep_core_inputs(k, slot_gid, region_seq, lvl, oy, ox, Wbf)
        im = {f"fcl{l}": feats_cl[l] for l in range(4)}
        im["wmat"] = wm
        im["idx"] = idx_tile
        in_maps.append(im)

    trace = os.environ.get("KERNEL_TRACE", "0") == "1"
    res = run_bass_kernel_spmd(nc, in_maps, list(range(N_CORES)), trace=trace)
    LAST_EXEC_TIME_NS = res.exec_time_ns

    out_full = np.zeros((N, C, S2), dtype=np.float32)
    for k in range(N_CORES):
        out_full[slot_gid[k]] = res.results[k]["out"].transpose(1, 0, 2)
    return out_full.reshape(N, C, S, S)


# revision 6
# speedup vs baseline: 2.5884x; 1.2372x over previous
"""Feature-pyramid ROIAlign (multi-level crop) on 8 TRN2 NeuronCores — v4.

Host routes (level assignment, 11x11 windows, bf16 bilinear weight
matrices); the device gathers each proposal's [11,11,256] bf16 patch from
a channels-last feature arena into a cell-on-partition SBUF tile and
interpolates with two k=121 bf16 matmuls into PSUM, then writes the
c-major f32 output shard.

Patch gathers are split across three engines to spread issue cost:
SP/ACT issue HWDGE DMAs with register-sourced dynamic offsets; GpSimd
issues per-proposal indirect DMAs (one int32 cell-row index per
partition). All 8 cores share one SPMD graph; per-core differences are
pure data (offset tables, weights).
"""
import os
import numpy as np
import ml_dtypes

RPN_SCALES = (2.0, 4.0, 8.0, 16.0)
BASE_SIZES = (8.0, 16.0, 32.0, 64.0)
S = 14
S2 = S * S
PW = 11
CELLS = PW * PW
C = 256
MAP_HW = (256, 128, 64, 32)
ARENA_BASE = (0, 65536, 81920, 86016)  # cell-row base of each level
ARENA_ROWS = 87040
N_CORES = 8
GRP = 8
# per-group engine pattern: 0=SP(dyn), 1=ACT(dyn), 2=GpSimd(indirect)
ENG_PATTERN = (2, 1, 0, 2, 1, 1, 2, 0)

LAST_EXEC_TIME_NS = None
_GRAPH_CACHE = {}


def _route_and_weights(proposals):
    p = proposals.astype(np.float32)
    x0, y0, x1, y1 = p[:, 1], p[:, 2], p[:, 3], p[:, 4]
    sizes = np.sqrt((x1 - x0) * (y1 - y0))
    base = np.asarray(BASE_SIZES, dtype=np.float32)
    dist = np.abs(sizes[:, None] - base[None, :])
    lvl = np.argmin(dist, axis=1).astype(np.int32)

    N = p.shape[0]
    stride = np.asarray(RPN_SCALES, dtype=np.float32)[lvl]
    M = np.asarray(MAP_HW, dtype=np.int32)[lvl]

    fx0, fy0, fx1, fy1 = (c / stride for c in (x0, y0, x1, y1))
    bw = (fx1 - fx0) / np.float32(S)
    bh = (fy1 - fy0) / np.float32(S)
    grid = np.arange(S, dtype=np.float32) + np.float32(0.5)
    xs = fx0[:, None] + grid[None, :] * bw[:, None] - np.float32(0.5)
    ys = fy0[:, None] + grid[None, :] * bh[:, None] - np.float32(0.5)

    def split(coord, Mv):
        c0 = np.floor(coord)
        frac = coord - c0
        i0 = np.clip(c0.astype(np.int64), 0, Mv - 1).astype(np.int32)
        i1 = np.minimum(i0 + 1, Mv - 1).astype(np.int32)
        return i0, i1, frac.astype(np.float32)

    Mv = M[:, None]
    yi0, yi1, wy = split(ys, Mv)
    xi0, xi1, wx = split(xs, Mv)

    oy = np.clip(yi0.min(axis=1), 0, M - PW)
    ox = np.clip(xi0.min(axis=1), 0, M - PW)
    ly0, ly1 = yi0 - oy[:, None], yi1 - oy[:, None]
    lx0, lx1 = xi0 - ox[:, None], xi1 - ox[:, None]
    assert ly0.min() >= 0 and lx0.min() >= 0 and ly1.max() < PW and lx1.max() < PW, \
        "proposal spans >11 feature cells; patch window too small"

    ii = np.arange(S)
    nn = np.arange(N)[:, None]
    Wy = np.zeros((N, S, PW), dtype=np.float32)
    Wx = np.zeros((N, S, PW), dtype=np.float32)
    np.add.at(Wy, (nn, ii[None, :], ly0), 1.0 - wy)
    np.add.at(Wy, (nn, ii[None, :], ly1), wy)
    np.add.at(Wx, (nn, ii[None, :], lx0), 1.0 - wx)
    np.add.at(Wx, (nn, ii[None, :], lx1), wx)
    Wfull = np.einsum("niy,njx->nyxij", Wy, Wx).reshape(N, CELLS, S2)
    return lvl, oy.astype(np.int32), ox.astype(np.int32), Wfull


def _shard(lvl):
    slot_gid = [[] for _ in range(N_CORES)]
    level_seq = []
    for l in range(4):
        ids = np.where(lvl == l)[0]
        if len(ids) == 0:
            continue
        pad = (-len(ids)) % N_CORES
        ids = np.concatenate([ids, np.repeat(ids[-1], pad)])
        per = len(ids) // N_CORES
        for k in range(N_CORES):
            slot_gid[k].extend(ids[k::N_CORES].tolist())
        level_seq.extend([l] * per)
    return (np.asarray(slot_gid, dtype=np.int64),
            np.asarray(level_seq, dtype=np.int64))


def _slot_engines(M):
    """Engine id per slot, and per-engine orderings."""
    eng = [ENG_PATTERN[j - (j // GRP) * GRP] for j in range(M)]
    sp = [j for j in range(M) if eng[j] == 0]
    act = [j for j in range(M) if eng[j] == 1]
    q7 = [j for j in range(M) if eng[j] == 2]
    return np.asarray(eng), sp, act, q7


def _build_graph(level_seq):
    import concourse.bass as bass
    import concourse.bacc as bacc
    import concourse.mybir as mybir
    import concourse.tile as tile

    M = len(level_seq)
    eng, sp_slots, act_slots, q7_slots = _slot_engines(M)
    n_sp, n_act, n_q7 = len(sp_slots), len(act_slots), len(q7_slots)
    sp_pos = {j: i for i, j in enumerate(sp_slots)}
    act_pos = {j: i for i, j in enumerate(act_slots)}
    q7_pos = {j: i for i, j in enumerate(q7_slots)}

    SP = (mybir.EngineType.SP,)
    ACT = (mybir.EngineType.Activation,)
    nc = bacc.Bacc()
    arena = nc.declare_dram_parameter("arena", [ARENA_ROWS, C],
                                      mybir.dt.bfloat16, isOutput=False)
    lvl_view = [arena[ARENA_BASE[l]:ARENA_BASE[l] + MAP_HW[l] * MAP_HW[l], :]
                for l in range(4)]
    wmat = nc.declare_dram_parameter("wmat", [CELLS, M, S2], mybir.dt.bfloat16,
                                     isOutput=False)
    # fused (oy*W+ox) offsets for SP slots then ACT slots (level-relative)
    orig = nc.declare_dram_parameter("orig", [1, max(n_sp + n_act, 1)],
                                     mybir.dt.int32, isOutput=False)
    # absolute arena cell ids for Q7 slots, [121, n_q7]
    idxg = nc.declare_dram_parameter("idxg", [CELLS, max(n_q7, 1)],
                                     mybir.dt.int32, isOutput=False)
    out = nc.declare_dram_parameter("out", [C, M, S2], mybir.dt.float32,
                                    isOutput=True)

    n_groups = (M + GRP - 1) // GRP
    with tile.TileContext(nc) as tc:
        with (
            tc.tile_pool(name="small", bufs=1) as psmall,
            tc.tile_pool(name="wpool", bufs=3) as pwp,
            tc.tile_pool(name="patch", bufs=24) as pp,
            tc.tile_pool(name="outp", bufs=3) as po,
            tc.tile_pool(name="ps", bufs=4, space="PSUM") as ppsum,
        ):
            orig_t = psmall.tile([1, max(n_sp + n_act, 1)], mybir.dt.int32)
            nc.sync.dma_start(orig_t[:], orig[:])
            idxg_t = psmall.tile([CELLS, max(n_q7, 1)], mybir.dt.int32)
            nc.sync.dma_start(idxg_t[:], idxg[:])
            sp_used = act_used = 0
            for g in range(n_groups):
                a = g * GRP
                b = min(a + GRP, M)
                grp = b - a
                wt = pwp.tile([CELLS, grp * S2], mybir.dt.bfloat16, tag="wt")
                nc.sync.dma_start(
                    wt[:].rearrange("k (p n) -> k p n", p=grp),
                    wmat[:, a:b, :])
                # batched register loads for this group's SP/ACT slots
                g_sp = [j for j in range(a, b) if eng[j] == 0]
                g_act = [j for j in range(a, b) if eng[j] == 1]
                vals_sp = vals_act = ()
                if g_sp:
                    o = sp_pos[g_sp[0]]
                    _, vals_sp = nc.values_load_multi_w_load_instructions(
                        orig_t[0:1, o:o + len(g_sp)], engines=SP,
                        skip_runtime_bounds_check=True)
                if g_act:
                    o = n_sp + act_pos[g_act[0]]
                    _, vals_act = nc.values_load_multi_w_load_instructions(
                        orig_t[0:1, o:o + len(g_act)], engines=ACT,
                        skip_runtime_bounds_check=True)
                pts = []
                for q in range(grp):
                    j = a + q
                    l = level_seq[j]
                    Wl = MAP_HW[l]
                    pt = pp.tile([CELLS, C], mybir.dt.bfloat16, tag="pt")
                    if eng[j] == 2:
                        nc.gpsimd.indirect_dma_start(
                            out=pt[:],
                            out_offset=None,
                            in_=arena[:],
                            in_offset=bass.IndirectOffsetOnAxis(
                                ap=idxg_t[:, q7_pos[j]:q7_pos[j] + 1], axis=0),
                        )
                    else:
                        if eng[j] == 0:
                            e = nc.sync
                            comb = vals_sp[g_sp.index(j)]
                        else:
                            e = nc.scalar
                            comb = vals_act[g_act.index(j)]
                        src = (lvl_view[l][bass.ds(comb, PW * Wl), :]
                               .rearrange("(y x) c -> y x c", x=Wl)[:, 0:PW, :])
                        e.dma_start(pt[:], src)
                    pts.append(pt)
                outA = po.tile([128, grp * S2], mybir.dt.float32, tag="outA")
                outB = po.tile([128, grp * S2], mybir.dt.float32, tag="outB")
                for q0 in range(0, grp, 2):
                    pair = min(2, grp - q0)
                    psA = ppsum.tile([128, pair * S2], mybir.dt.float32, tag="psA")
                    psB = ppsum.tile([128, pair * S2], mybir.dt.float32, tag="psB")
                    for dq in range(pair):
                        q = q0 + dq
                        sl_w = slice(q * S2, (q + 1) * S2)
                        sl_p = slice(dq * S2, (dq + 1) * S2)
                        nc.tensor.matmul(psA[:, sl_p], pts[q][:, 0:128],
                                         wt[:, sl_w], start=True, stop=True)
                        nc.tensor.matmul(psB[:, sl_p], pts[q][:, 128:256],
                                         wt[:, sl_w], start=True, stop=True)
                    sl_o = slice(q0 * S2, (q0 + pair) * S2)
                    nc.vector.tensor_copy(outA[:, sl_o], psA[:])
                    nc.vector.tensor_copy(outB[:, sl_o], psB[:])
                nc.sync.dma_start(out[0:128, a:b, :], outA[:])
                nc.scalar.dma_start(out[128:256, a:b, :], outB[:])
    nc.finalize()
    return nc


def _prep_core_inputs(k, slot_gid, level_seq, lvl, oy, ox, Wbf):
    M = slot_gid.shape[1]
    eng, sp_slots, act_slots, q7_slots = _slot_engines(M)
    gids = slot_gid[k]
    wm = np.ascontiguousarray(Wbf[gids].transpose(1, 0, 2))  # [121, M, 196]

    Wl = np.asarray(MAP_HW)[lvl[gids]].astype(np.int64)
    comb = oy[gids].astype(np.int64) * Wl + ox[gids]  # level-relative
    og = np.concatenate([comb[sp_slots], comb[act_slots]])
    og = np.ascontiguousarray(og.reshape(1, -1).astype(np.int32))
    if og.size == 0:
        og = np.zeros((1, 1), np.int32)

    dy = np.repeat(np.arange(PW), PW)
    dx = np.tile(np.arange(PW), PW)
    ig = np.zeros((CELLS, max(len(q7_slots), 1)), np.int64)
    for i, j in enumerate(q7_slots):
        g = gids[j]
        W = MAP_HW[lvl[g]]
        ig[:, i] = (ARENA_BASE[lvl[g]] + (oy[g] + dy) * W + (ox[g] + dx))
    idxg = np.ascontiguousarray(ig.astype(np.int32))
    return wm, og, idxg


def kernel(f0, f1, f2, f3, proposals):
    global LAST_EXEC_TIME_NS
    try:
        import profile_hook
        profile_hook.install()
    except Exception:
        pass
    from concourse.bass_utils import run_bass_kernel_spmd

    feats = (f0, f1, f2, f3)
    N = proposals.shape[0]
    lvl, oy, ox, Wfull = _route_and_weights(np.asarray(proposals))
    slot_gid, level_seq = _shard(lvl)
    M = slot_gid.shape[1]

    key = tuple(level_seq.tolist())
    if key not in _GRAPH_CACHE:
        _GRAPH_CACHE[key] = _build_graph(level_seq)
    nc = _GRAPH_CACHE[key]

    arena_np = np.concatenate([
        np.ascontiguousarray(np.asarray(f)[0].transpose(1, 2, 0)).astype(
            ml_dtypes.bfloat16).reshape(-1, C)
        for f in feats
    ], axis=0)
    assert arena_np.shape[0] == ARENA_ROWS
    Wbf = Wfull.astype(ml_dtypes.bfloat16)

    in_maps = []
    for k in range(N_CORES):
        wm, og, idxg = _prep_core_inputs(k, slot_gid, level_seq, lvl, oy, ox, Wbf)
        in_maps.append({"arena": arena_np, "wmat": wm, "orig": og, "idxg": idxg})

    trace = os.environ.get("KERNEL_TRACE", "0") == "1"
    res = run_bass_kernel_spmd(nc, in_maps, list(range(N_CORES)), trace=trace)
    LAST_EXEC_TIME_NS = res.exec_time_ns

    out_full = np.zeros((N, C, S2), dtype=np.float32)
    for k in range(N_CORES):
        out_full[slot_gid[k]] = res.results[k]["out"].transpose(1, 0, 2)
    return out_full.reshape(N, C, S, S)


# revision 8
# speedup vs baseline: 3.3051x; 1.2769x over previous
"""Feature-pyramid ROIAlign (multi-level crop) on 8 TRN2 NeuronCores — v4.

Host routes (level assignment, 11x11 windows, bf16 bilinear weight
matrices); the device gathers each proposal's [11,11,256] bf16 patch from
a channels-last feature arena into a cell-on-partition SBUF tile and
interpolates with two k=121 bf16 matmuls into PSUM, then writes the
c-major f32 output shard.

Patch gathers are split across three engines to spread issue cost:
SP/ACT issue HWDGE DMAs with register-sourced dynamic offsets; GpSimd
issues per-proposal indirect DMAs (one int32 cell-row index per
partition). All 8 cores share one SPMD graph; per-core differences are
pure data (offset tables, weights).
"""
import os
import numpy as np
import ml_dtypes

RPN_SCALES = (2.0, 4.0, 8.0, 16.0)
BASE_SIZES = (8.0, 16.0, 32.0, 64.0)
S = 14
S2 = S * S
PW = 11
CELLS = PW * PW
C = 256
MAP_HW = (256, 128, 64, 32)
ARENA_BASE = (0, 65536, 81920, 86016)  # cell-row base of each level
ARENA_ROWS = 87040
N_CORES = 8
GRP = 8
# per-group engine pattern: 0=SP(dyn), 1=ACT(dyn), 2=GpSimd(indirect)
ENG_PATTERN = (2, 1, 0, 2, 1, 2, 2, 0)

LAST_EXEC_TIME_NS = None
_GRAPH_CACHE = {}


def _route_and_weights(proposals):
    p = proposals.astype(np.float32)
    x0, y0, x1, y1 = p[:, 1], p[:, 2], p[:, 3], p[:, 4]
    sizes = np.sqrt((x1 - x0) * (y1 - y0))
    base = np.asarray(BASE_SIZES, dtype=np.float32)
    dist = np.abs(sizes[:, None] - base[None, :])
    lvl = np.argmin(dist, axis=1).astype(np.int32)

    N = p.shape[0]
    stride = np.asarray(RPN_SCALES, dtype=np.float32)[lvl]
    M = np.asarray(MAP_HW, dtype=np.int32)[lvl]

    fx0, fy0, fx1, fy1 = (c / stride for c in (x0, y0, x1, y1))
    bw = (fx1 - fx0) / np.float32(S)
    bh = (fy1 - fy0) / np.float32(S)
    grid = np.arange(S, dtype=np.float32) + np.float32(0.5)
    xs = fx0[:, None] + grid[None, :] * bw[:, None] - np.float32(0.5)
    ys = fy0[:, None] + grid[None, :] * bh[:, None] - np.float32(0.5)

    def split(coord, Mv):
        c0 = np.floor(coord)
        frac = coord - c0
        i0 = np.clip(c0.astype(np.int64), 0, Mv - 1).astype(np.int32)
        i1 = np.minimum(i0 + 1, Mv - 1).astype(np.int32)
        return i0, i1, frac.astype(np.float32)

    Mv = M[:, None]
    yi0, yi1, wy = split(ys, Mv)
    xi0, xi1, wx = split(xs, Mv)

    oy = np.clip(yi0.min(axis=1), 0, M - PW)
    ox = np.clip(xi0.min(axis=1), 0, M - PW)
    ly0, ly1 = yi0 - oy[:, None], yi1 - oy[:, None]
    lx0, lx1 = xi0 - ox[:, None], xi1 - ox[:, None]
    assert ly0.min() >= 0 and lx0.min() >= 0 and ly1.max() < PW and lx1.max() < PW, \
        "proposal spans >11 feature cells; patch window too small"

    ii = np.arange(S)
    nn = np.arange(N)[:, None]
    Wy = np.zeros((N, S, PW), dtype=np.float32)
    Wx = np.zeros((N, S, PW), dtype=np.float32)
    np.add.at(Wy, (nn, ii[None, :], ly0), 1.0 - wy)
    np.add.at(Wy, (nn, ii[None, :], ly1), wy)
    np.add.at(Wx, (nn, ii[None, :], lx0), 1.0 - wx)
    np.add.at(Wx, (nn, ii[None, :], lx1), wx)
    Wfull = np.einsum("niy,njx->nyxij", Wy, Wx).reshape(N, CELLS, S2)
    return lvl, oy.astype(np.int32), ox.astype(np.int32), Wfull


def _shard(lvl):
    slot_gid = [[] for _ in range(N_CORES)]
    level_seq = []
    for l in range(4):
        ids = np.where(lvl == l)[0]
        if len(ids) == 0:
            continue
        pad = (-len(ids)) % N_CORES
        ids = np.concatenate([ids, np.repeat(ids[-1], pad)])
        per = len(ids) // N_CORES
        for k in range(N_CORES):
            slot_gid[k].extend(ids[k::N_CORES].tolist())
        level_seq.extend([l] * per)
    return (np.asarray(slot_gid, dtype=np.int64),
            np.asarray(level_seq, dtype=np.int64))


def _slot_engines(M):
    """Engine id per slot, and per-engine orderings."""
    eng = [ENG_PATTERN[j - (j // GRP) * GRP] for j in range(M)]
    sp = [j for j in range(M) if eng[j] == 0]
    act = [j for j in range(M) if eng[j] == 1]
    q7 = [j for j in range(M) if eng[j] == 2]
    return np.asarray(eng), sp, act, q7


def _build_graph(level_seq):
    import concourse.bass as bass
    import concourse.bacc as bacc
    import concourse.mybir as mybir
    import concourse.tile as tile

    M = len(level_seq)
    eng, sp_slots, act_slots, q7_slots = _slot_engines(M)
    n_sp, n_act, n_q7 = len(sp_slots), len(act_slots), len(q7_slots)
    sp_pos = {j: i for i, j in enumerate(sp_slots)}
    act_pos = {j: i for i, j in enumerate(act_slots)}
    q7_pos = {j: i for i, j in enumerate(q7_slots)}

    SP = (mybir.EngineType.SP,)
    ACT = (mybir.EngineType.Activation,)
    nc = bacc.Bacc()
    arena = nc.declare_dram_parameter("arena", [ARENA_ROWS, C],
                                      mybir.dt.bfloat16, isOutput=False)
    lvl_view = [arena[ARENA_BASE[l]:ARENA_BASE[l] + MAP_HW[l] * MAP_HW[l], :]
                for l in range(4)]
    wmat = nc.declare_dram_parameter("wmat", [CELLS, M, S2], mybir.dt.bfloat16,
                                     isOutput=False)
    # fused (oy*W+ox) offsets for SP slots then ACT slots (level-relative)
    orig = nc.declare_dram_parameter("orig", [1, max(n_sp + n_act, 1)],
                                     mybir.dt.int32, isOutput=False)
    # absolute arena cell ids for Q7 slots, [121, n_q7]
    idxg = nc.declare_dram_parameter("idxg", [CELLS, max(n_q7, 1)],
                                     mybir.dt.int32, isOutput=False)
    out = nc.declare_dram_parameter("out", [C, M, S2], mybir.dt.bfloat16,
                                    isOutput=True)

    n_groups = (M + GRP - 1) // GRP
    with tile.TileContext(nc) as tc:
        with (
            tc.tile_pool(name="small", bufs=1) as psmall,
            tc.tile_pool(name="wpool", bufs=4) as pwp,
            tc.tile_pool(name="patch", bufs=32) as pp,
            tc.tile_pool(name="outp", bufs=4) as po,
            tc.tile_pool(name="ps", bufs=4, space="PSUM") as ppsum,
        ):
            orig_t = psmall.tile([1, max(n_sp + n_act, 1)], mybir.dt.int32)
            nc.sync.dma_start(orig_t[:], orig[:])
            idxg_t = psmall.tile([CELLS, max(n_q7, 1)], mybir.dt.int32)
            nc.sync.dma_start(idxg_t[:], idxg[:])
            sp_used = act_used = 0
            for g in range(n_groups):
                a = g * GRP
                b = min(a + GRP, M)
                grp = b - a
                wt = pwp.tile([CELLS, grp * S2], mybir.dt.bfloat16, tag="wt")
                nc.sync.dma_start(
                    wt[:].rearrange("k (p n) -> k p n", p=grp),
                    wmat[:, a:b, :])
                # batched register loads for this group's SP/ACT slots
                g_sp = [j for j in range(a, b) if eng[j] == 0]
                g_act = [j for j in range(a, b) if eng[j] == 1]
                vals_sp = vals_act = ()
                if g_sp:
                    o = sp_pos[g_sp[0]]
                    _, vals_sp = nc.values_load_multi_w_load_instructions(
                        orig_t[0:1, o:o + len(g_sp)], engines=SP,
                        skip_runtime_bounds_check=True)
                if g_act:
                    o = n_sp + act_pos[g_act[0]]
                    _, vals_act = nc.values_load_multi_w_load_instructions(
                        orig_t[0:1, o:o + len(g_act)], engines=ACT,
                        skip_runtime_bounds_check=True)
                pts = []
                for q in range(grp):
                    j = a + q
                    l = level_seq[j]
                    Wl = MAP_HW[l]
                    pt = pp.tile([CELLS, C], mybir.dt.bfloat16, tag="pt")
                    if eng[j] == 2:
                        nc.gpsimd.indirect_dma_start(
                            out=pt[:],
                            out_offset=None,
                            in_=arena[:],
                            in_offset=bass.IndirectOffsetOnAxis(
                                ap=idxg_t[:, q7_pos[j]:q7_pos[j] + 1], axis=0),
                        )
                    else:
                        if eng[j] == 0:
                            e = nc.sync
                            comb = vals_sp[g_sp.index(j)]
                        else:
                            e = nc.scalar
                            comb = vals_act[g_act.index(j)]
                        src = (lvl_view[l][bass.ds(comb, PW * Wl), :]
                               .rearrange("(y x) c -> y x c", x=Wl)[:, 0:PW, :])
                        e.dma_start(pt[:], src)
                    pts.append(pt)
                outA = po.tile([128, grp * S2], mybir.dt.bfloat16, tag="outA")
                outB = po.tile([128, grp * S2], mybir.dt.bfloat16, tag="outB")
                for q0 in range(0, grp, 2):
                    pair = min(2, grp - q0)
                    psA = ppsum.tile([128, pair * S2], mybir.dt.float32, tag="psA")
                    psB = ppsum.tile([128, pair * S2], mybir.dt.float32, tag="psB")
                    for dq in range(pair):
                        q = q0 + dq
                        sl_w = slice(q * S2, (q + 1) * S2)
                        sl_p = slice(dq * S2, (dq + 1) * S2)
                        nc.tensor.matmul(psA[:, sl_p], pts[q][:, 0:128],
                                         wt[:, sl_w], start=True, stop=True)
                        nc.tensor.matmul(psB[:, sl_p], pts[q][:, 128:256],
                                         wt[:, sl_w], start=True, stop=True)
                    sl_o = slice(q0 * S2, (q0 + pair) * S2)
                    nc.vector.tensor_copy(outA[:, sl_o], psA[:])
                    nc.vector.tensor_copy(outB[:, sl_o], psB[:])
                nc.sync.dma_start(out[0:128, a:b, :], outA[:])
                nc.scalar.dma_start(out[128:256, a:b, :], outB[:])
    nc.finalize()
    return nc


def _prep_core_inputs(k, slot_gid, level_seq, lvl, oy, ox, Wbf):
    M = slot_gid.shape[1]
    eng, sp_slots, act_slots, q7_slots = _slot_engines(M)
    gids = slot_gid[k]
    wm = np.ascontiguousarray(Wbf[gids].transpose(1, 0, 2))  # [121, M, 196]

    Wl = np.asarray(MAP_HW)[lvl[gids]].astype(np.int64)
    comb = oy[gids].astype(np.int64) * Wl + ox[gids]  # level-relative
    og = np.concatenate([comb[sp_slots], comb[act_slots]])
    og = np.ascontiguousarray(og.reshape(1, -1).astype(np.int32))
    if og.size == 0:
        og = np.zeros((1, 1), np.int32)

    dy = np.repeat(np.arange(PW), PW)
    dx = np.tile(np.arange(PW), PW)
    ig = np.zeros((CELLS, max(len(q7_slots), 1)), np.int64)
    for i, j in enumerate(q7_slots):
        g = gids[j]
        W = MAP_HW[lvl[g]]
        ig[:, i] = (ARENA_BASE[lvl[g]] + (oy[g] + dy) * W + (ox[g] + dx))
    idxg = np.ascontiguousarray(ig.astype(np.int32))
    return wm, og, idxg


def kernel(f0, f1, f2, f3, proposals):
    global LAST_EXEC_TIME_NS
    try:
        import profile_hook
        profile_hook.install()
    except Exception:
        pass
    from concourse.bass_utils import run_bass_kernel_spmd

    feats = (f0, f1, f2, f3)
    N = proposals.shape[0]
    lvl, oy, ox, Wfull = _route_and_weights(np.asarray(proposals))
    slot_gid, level_seq = _shard(lvl)
    M = slot_gid.shape[1]

    key = tuple(level_seq.tolist())
    if key not in _GRAPH_CACHE:
        _GRAPH_CACHE[key] = _build_graph(level_seq)
    nc = _GRAPH_CACHE[key]

    arena_np = np.concatenate([
        np.ascontiguousarray(np.asarray(f)[0].transpose(1, 2, 0)).astype(
            ml_dtypes.bfloat16).reshape(-1, C)
        for f in feats
    ], axis=0)
    assert arena_np.shape[0] == ARENA_ROWS
    Wbf = Wfull.astype(ml_dtypes.bfloat16)

    in_maps = []
    for k in range(N_CORES):
        wm, og, idxg = _prep_core_inputs(k, slot_gid, level_seq, lvl, oy, ox, Wbf)
        in_maps.append({"arena": arena_np, "wmat": wm, "orig": og, "idxg": idxg})

    trace = os.environ.get("KERNEL_TRACE", "0") == "1"
    res = run_bass_kernel_spmd(nc, in_maps, list(range(N_CORES)), trace=trace)
    LAST_EXEC_TIME_NS = res.exec_time_ns

    out_full = np.zeros((N, C, S2), dtype=np.float32)
    for k in range(N_CORES):
        out_full[slot_gid[k]] = res.results[k]["out"].astype(np.float32).transpose(1, 0, 2)
    return out_full.reshape(N, C, S, S)
